# revision 22
# baseline (speedup 1.0000x reference)
"""Trainium2 Bass kernel for nn_BayesianFlowNetworkDiscretised.

Computes, for each (b, d) position:
    MLP: h = gelu_tanh(W1[0,:]*mu + t*W1[1,:] + b1); (mu_eps, ln_sig) = h@W2 + b2
    mu_x = mu/gamma - var_scale*mu_eps
    sigma = max(var_scale*exp(ln_sig), 0.02)   [clip never binds for this data]
    out_k = Phi((e_k - mu_x)/sigma) - Phi((e_{k-1} - mu_x)/sigma),  e_i = i/8 - 1

Sharding: D split across 8 cores (data-parallel, no comm).
Per-core layout: partition p = b*4 + q holds mu[b, q*1536 : (q+1)*1536];
all per-b constants become per-partition [128,1] scale/bias vectors.

dtypes: fp16 for h / MLP accumulators / inv / erf outputs (error-analysed
safe: beta*inv <= ~1 bounds amplification); fp32 for mu, mu_x, final out.
"""

import sys

sys.path.insert(0, "/opt/trn_rl_repo")

import numpy as np

import concourse.bass as bass
import concourse.bacc as bacc
from concourse import mybir
from concourse.tile import TileContext
from concourse.bass_utils import run_bass_kernel_spmd

F32 = mybir.dt.float32
F16 = mybir.dt.float16
AF = mybir.ActivationFunctionType
OP = mybir.AluOpType

K = 16
SIGMA_ONE = 0.02
T_MIN = 1e-6
B, D, H = 32, 49152, 16
NCORES = 8
DS = D // NCORES          # 6144 columns per core
Q = 4                     # partitions per batch row
F = DS // Q               # 1536 free elements per partition
NCHUNK = 2                # output staging chunks
FC = F // NCHUNK          # 512
LN_SQRT2 = 0.34657359027997264


def _build(W1, b1, W2, b2):
    """Build the Bass module. Weights are baked in as immediates.

    The shard is processed in two column-halves forming a 2-stage software
    pipeline: half-2's gelu phase (ACT-heavy) overlaps half-1's
    args/erf/diffs phase (DVE-heavy).
    """
    nc = bacc.Bacc(None, target_bir_lowering=False)
    mu_p = nc.declare_dram_parameter("mu", [B, DS], F32, isOutput=False)
    cn_p = nc.declare_dram_parameter("cn", [128, H + 8], F32, isOutput=False)
    out_p = nc.declare_dram_parameter("out", [128, K, F], F32, isOutput=True)

    mu_v = mu_p.rearrange("b (q f) -> (b q) f", q=Q)
    HF = F // 2

    with TileContext(nc) as tc:
        with (
            tc.tile_pool(name="const", bufs=1) as constp,
            tc.tile_pool(name="main", bufs=1) as mainp,
            tc.tile_pool(name="tp", bufs=2) as tpool,
            tc.tile_pool(name="ph", bufs=2) as php,
            tc.tile_pool(name="hp", bufs=4) as hp,
            tc.tile_pool(name="fp", bufs=18) as fpool,
            tc.tile_pool(name="op", bufs=6) as opool,
        ):
            cn = constp.tile([128, H + 8], F32)
            nc.sync.dma_start(out=cn[:, :], in_=cn_p[:, :])
            cb = cn[:, 0:H]
            pb = cn[:, H : H + 8]
            mu = mainp.tile([128, F], F32)
            nc.sync.dma_start(out=mu[:, :], in_=mu_v)

            # ACT instructions support a single sync-wait slot; make the ACT
            # engine observe the const-DMA semaphore via a tiny copy so the
            # first gelu only needs to wait on the mu DMA.
            warm = constp.tile([128, 1], F32)
            nc.scalar.copy(out=warm[:, :], in_=cn[:, 0:1])

            alpha = pb[:, 0:1]      # 1/gamma            (0 if cond)
            negbeta = pb[:, 1:2]    # -var_scale         (0 if cond)
            lnA = pb[:, 2:3]        # ln(var_scale)      (-1e4 if cond)
            lnm = pb[:, 3:4]        # ln(0.02)           (0 if cond)

            T_es, T_ls, invs, mxs = [], [], [], []
            NACT_E = 11   # e-col muls moved to ACT (engine balance)

            # loop 1: gelu + W2-scaled slot fills, both halves
            for hf in range(2):
                sl = slice(hf * HF, (hf + 1) * HF)
                muh = mu[:, sl]
                T_e = tpool.tile([128, H, HF], F16)
                T_l = tpool.tile([128, H, HF], F16)
                T_es.append(T_e); T_ls.append(T_l)
                for j in range(H):
                    h = hp.tile([128, HF], F16)
                    nc.scalar.activation(
                        out=h, in_=muh, func=AF.Gelu_apprx_tanh,
                        bias=cb[:, j : j + 1], scale=float(W1[0, j]),
                    )
                    if j == 0:
                        nc.vector.tensor_scalar(
                            out=T_e[:, j, :], in0=h, scalar1=float(W2[j, 0]),
                            scalar2=float(b2[0]), op0=OP.mult, op1=OP.add)
                        nc.vector.tensor_scalar(
                            out=T_l[:, j, :], in0=h, scalar1=float(W2[j, 1]),
                            scalar2=float(b2[1]), op0=OP.mult, op1=OP.add)
                    else:
                        if j > H - 1 - NACT_E:
                            nc.scalar.activation(
                                out=T_e[:, j, :], in_=h, func=AF.Copy,
                                scale=float(W2[j, 0]))
                        else:
                            nc.vector.tensor_scalar_mul(
                                out=T_e[:, j, :], in0=h, scalar1=float(W2[j, 0]))
                        nc.vector.tensor_scalar_mul(
                            out=T_l[:, j, :], in0=h, scalar1=float(W2[j, 1]))

            # loop 2a: l-tree + v + exp for both halves (exp gates the rest)
            for hf in range(2):
                T_l = T_ls[hf]
                for w in (8, 4, 2, 1):
                    nc.vector.tensor_tensor(
                        out=T_l[:, 0:w, :], in0=T_l[:, 0:w, :],
                        in1=T_l[:, w : 2 * w, :], op=OP.add)
                v = php.tile([128, HF], F16)
                nc.vector.tensor_scalar(
                    out=v, in0=T_l[:, 0, :], scalar1=lnA, scalar2=lnm,
                    op0=OP.add, op1=OP.max)
                inv = php.tile([128, HF], F16)
                nc.scalar.activation(
                    out=inv, in_=v, func=AF.Exp, scale=-1.0, bias=pb[:, 5:6])
                invs.append(inv)

            # loop 2b: e-tree, mu_x, args, erf, diffs per half
            for hf in range(2):
                sl = slice(hf * HF, (hf + 1) * HF)
                muh = mu[:, sl]
                T_e = T_es[hf]
                inv = invs[hf]
                for w in (8, 4, 2, 1):
                    nc.vector.tensor_tensor(
                        out=T_e[:, 0:w, :], in0=T_e[:, 0:w, :],
                        in1=T_e[:, w : 2 * w, :], op=OP.add)
                mx = php.tile([128, HF], F32)
                nc.vector.tensor_scalar_mul(out=mx, in0=muh, scalar1=alpha)
                nc.vector.scalar_tensor_tensor(
                    out=mx, in0=T_e[:, 0, :], scalar=negbeta, in1=mx,
                    op0=OP.mult, op1=OP.add)

                st = php.tile([128, HF], F16)
                nc.vector.tensor_scalar_mul(out=st, in0=inv, scalar1=0.125)
                ats = {}
                for i in (3, 8, 13):
                    a = fpool.tile([128, HF], F16)
                    nc.vector.tensor_scalar(
                        out=a, in0=mx, scalar1=-1.0,
                        scalar2=float(i / 8.0 - 1.0), op0=OP.mult, op1=OP.add)
                    nc.vector.tensor_mul(out=a, in0=a, in1=inv)
                    ats[i] = a
                for src_i, dst_i in ((3, 2), (2, 1), (3, 4), (4, 5),
                                     (8, 7), (7, 6), (8, 9), (9, 10),
                                     (13, 12), (12, 11), (13, 14), (14, 15)):
                    a = fpool.tile([128, HF], F16)
                    nc.vector.tensor_tensor(
                        out=a, in0=ats[src_i], in1=st,
                        op=OP.add if dst_i > src_i else OP.subtract)
                    ats[dst_i] = a
                fts = []
                for i in range(1, 16):
                    fi = ats[i]
                    nc.scalar.activation(out=fi, in_=fi, func=AF.Erf)
                    nc.vector.tensor_scalar_mul(out=fi, in0=fi, scalar1=0.5)
                    fts.append(fi)
                for k2 in range(K):
                    o = opool.tile([128, HF], F16)
                    if k2 == 0:
                        nc.vector.tensor_scalar_add(
                            out=o, in0=fts[0], scalar1=0.5)
                    elif k2 == 15:
                        nc.vector.tensor_scalar(
                            out=o, in0=fts[14], scalar1=-1.0, scalar2=0.5,
                            op0=OP.mult, op1=OP.add)
                    else:
                        nc.vector.tensor_tensor(
                            out=o, in0=fts[k2], in1=fts[k2 - 1],
                            op=OP.subtract)
                    nc.gpsimd.dma_start(out=out_p[:, k2, sl], in_=o[:, :])

    return nc


def _host_consts(t, W1, b1, W2, b2):
    t = np.asarray(t, np.float64).reshape(B)
    cond = t < T_MIN
    gamma = 1.0 - SIGMA_ONE ** (2.0 * t)
    alpha = np.where(cond, 0.0, 1.0 / gamma)
    beta = np.sqrt(np.maximum(1.0 - gamma, 0.0) / gamma)
    negbeta = np.where(cond, 0.0, -beta)
    lnA = np.where(cond, -1e4, np.log(np.maximum(beta, 1e-300)))
    lnm = np.where(cond, 0.0, np.log(SIGMA_ONE))
    nb20 = np.where(cond, 0.0, -beta * float(b2[0]))

    pb = np.zeros((128, 8), np.float32)
    for b in range(B):
        for q in range(Q):
            p = b * Q + q
            pb[p, 0] = alpha[b]
            pb[p, 1] = negbeta[b]
            pb[p, 2] = lnA[b]
            pb[p, 3] = lnm[b]
            pb[p, 4] = nb20[b]
            pb[p, 5] = -LN_SQRT2

    cb = np.zeros((128, H), np.float32)
    cvals = t[:, None] * np.asarray(W1, np.float64)[1, :][None, :] + np.asarray(
        b1, np.float64)[None, :]                        # [B, H]
    for b in range(B):
        cb[b * Q : (b + 1) * Q, :] = cvals[b]
    return cb, pb


def _run(inputs, trace=False):
    mu = np.ascontiguousarray(np.asarray(inputs["mu"], np.float32))
    t = np.asarray(inputs["t"], np.float32)
    W1 = np.asarray(inputs["W1"], np.float32)
    b1 = np.asarray(inputs["b1"], np.float32)
    W2 = np.asarray(inputs["W2"], np.float32)
    b2 = np.asarray(inputs["b2"], np.float32)

    nc = _build(W1, b1, W2, b2)
    nc.finalize()
    cb, pb = _host_consts(t, W1, b1, W2, b2)

    cn = np.ascontiguousarray(np.concatenate([cb, pb], axis=1))
    in_maps = []
    for c in range(NCORES):
        shard = np.ascontiguousarray(mu[:, c * DS : (c + 1) * DS])
        in_maps.append({"mu": shard, "cn": cn})

    res = run_bass_kernel_spmd(nc, in_maps, list(range(NCORES)), trace=trace)
    shards = []
    for c in range(NCORES):
        s = np.asarray(res.results[c]["out"])          # [128, K, F]
        shards.append(s.reshape(B, Q, K, F).transpose(0, 1, 3, 2).reshape(B, DS, K))
    out = np.ascontiguousarray(np.concatenate(shards, axis=1))
    return out, res


def kernel(**inputs) -> np.ndarray:
    out, _ = _run(inputs, trace=False)
    return out


if __name__ == "__main__":
    rng = np.random.default_rng(0)
    demo = {
        "mu": rng.standard_normal((B, D), dtype=np.float32),
        "t": rng.random((B, 1), dtype=np.float32),
        "W1": rng.standard_normal((2, H), dtype=np.float32) * 0.5,
        "b1": rng.standard_normal((H,), dtype=np.float32) * 0.1,
        "W2": rng.standard_normal((H, 2), dtype=np.float32) * 0.1,
        "b2": rng.standard_normal((2,), dtype=np.float32) * 0.1,
    }
    out = kernel(**demo)
    print("kernel output", out.shape, out.dtype, out[0, 0])


# revision 23
# speedup vs baseline: 1.0560x; 1.0560x over previous
"""Trainium2 Bass kernel for nn_BayesianFlowNetworkDiscretised.

Computes, for each (b, d) position:
    MLP: h = gelu_tanh(W1[0,:]*mu + t*W1[1,:] + b1); (mu_eps, ln_sig) = h@W2 + b2
    mu_x = mu/gamma - var_scale*mu_eps
    sigma = max(var_scale*exp(ln_sig), 0.02)   [clip never binds for this data]
    out_k = Phi((e_k - mu_x)/sigma) - Phi((e_{k-1} - mu_x)/sigma),  e_i = i/8 - 1

Sharding: D split across 8 cores (data-parallel, no comm).
Per-core layout: partition p = b*4 + q holds mu[b, q*1536 : (q+1)*1536];
all per-b constants become per-partition [128,1] scale/bias vectors.

dtypes: fp16 for h / MLP accumulators / inv / erf outputs (error-analysed
safe: beta*inv <= ~1 bounds amplification); fp32 for mu, mu_x, final out.
"""

import sys

sys.path.insert(0, "/opt/trn_rl_repo")

import numpy as np

import concourse.bass as bass
import concourse.bacc as bacc
from concourse import mybir
from concourse.tile import TileContext
from concourse.bass_utils import run_bass_kernel_spmd

F32 = mybir.dt.float32
F16 = mybir.dt.float16
AF = mybir.ActivationFunctionType
OP = mybir.AluOpType

K = 16
SIGMA_ONE = 0.02
T_MIN = 1e-6
B, D, H = 32, 49152, 16
NCORES = 8
DS = D // NCORES          # 6144 columns per core
Q = 4                     # partitions per batch row
F = DS // Q               # 1536 free elements per partition
NCHUNK = 2                # output staging chunks
FC = F // NCHUNK          # 512
LN_SQRT2 = 0.34657359027997264


def _build(W1, b1, W2, b2):
    """Build the Bass module. Weights are baked in as immediates.

    The shard is processed in two column-halves forming a 2-stage software
    pipeline: half-2's gelu phase (ACT-heavy) overlaps half-1's
    args/erf/diffs phase (DVE-heavy).
    """
    nc = bacc.Bacc(None, target_bir_lowering=False)
    mu_p = nc.declare_dram_parameter("mu", [B, DS], F32, isOutput=False)
    cn_p = nc.declare_dram_parameter("cn", [128, H + 8], F32, isOutput=False)
    out_p = nc.declare_dram_parameter("out", [128, K, F], F32, isOutput=True)

    mu_v = mu_p.rearrange("b (q f) -> (b q) f", q=Q)
    HF = F // 2

    with TileContext(nc) as tc:
        with (
            tc.tile_pool(name="const", bufs=1) as constp,
            tc.tile_pool(name="main", bufs=1) as mainp,
            tc.tile_pool(name="tp", bufs=2) as tpool,
            tc.tile_pool(name="ph", bufs=2) as php,
            tc.tile_pool(name="hp", bufs=4) as hp,
            tc.tile_pool(name="fp", bufs=18) as fpool,
            tc.tile_pool(name="op", bufs=6) as opool,
        ):
            cn = constp.tile([128, H + 8], F32)
            nc.sync.dma_start(out=cn[:, :], in_=cn_p[:, :])
            cb = cn[:, 0:H]
            pb = cn[:, H : H + 8]
            mu = mainp.tile([128, F], F32)
            nc.sync.dma_start(out=mu[:, :], in_=mu_v)

            # ACT instructions support a single sync-wait slot; make the ACT
            # engine observe the const-DMA semaphore via a tiny copy so the
            # first gelu only needs to wait on the mu DMA.
            warm = constp.tile([128, 1], F32)
            nc.scalar.copy(out=warm[:, :], in_=cn[:, 0:1])

            alpha = pb[:, 0:1]      # 1/gamma            (0 if cond)
            negbeta = pb[:, 1:2]    # -var_scale         (0 if cond)
            lnA = pb[:, 2:3]        # ln(var_scale)      (-1e4 if cond)
            lnm = pb[:, 3:4]        # ln(0.02)           (0 if cond)

            for hf in range(2):
                sl = slice(hf * HF, (hf + 1) * HF)
                muh = mu[:, sl]

                # ---- phase A: gelu on ACT; W2-scaled copies + tree adds on DVE
                T_e = tpool.tile([128, H, HF], F16)
                T_l = tpool.tile([128, H, HF], F16)
                for j in range(H):
                    h = hp.tile([128, HF], F16)
                    nc.scalar.activation(
                        out=h, in_=muh, func=AF.Gelu_apprx_tanh,
                        bias=cb[:, j : j + 1], scale=float(W1[0, j]),
                    )
                    if j == 0:
                        nc.vector.tensor_scalar(
                            out=T_e[:, j, :], in0=h, scalar1=float(W2[j, 0]),
                            scalar2=float(b2[0]), op0=OP.mult, op1=OP.add)
                        nc.vector.tensor_scalar(
                            out=T_l[:, j, :], in0=h, scalar1=float(W2[j, 1]),
                            scalar2=float(b2[1]), op0=OP.mult, op1=OP.add)
                    else:
                        nc.vector.tensor_scalar_mul(
                            out=T_e[:, j, :], in0=h, scalar1=float(W2[j, 0]))
                        nc.vector.tensor_scalar_mul(
                            out=T_l[:, j, :], in0=h, scalar1=float(W2[j, 1]))

                # l-column first: it gates exp/inv (the critical path).
                for w in (8, 4, 2, 1):
                    nc.vector.tensor_tensor(
                        out=T_l[:, 0:w, :], in0=T_l[:, 0:w, :],
                        in1=T_l[:, w : 2 * w, :], op=OP.add)
                v = php.tile([128, HF], F16)
                nc.vector.tensor_scalar(
                    out=v, in0=T_l[:, 0, :], scalar1=lnA, scalar2=lnm,
                    op0=OP.add, op1=OP.max)
                inv = php.tile([128, HF], F16)
                nc.scalar.activation(
                    out=inv, in_=v, func=AF.Exp, scale=-1.0, bias=pb[:, 5:6])

                for w in (8, 4, 2, 1):
                    nc.vector.tensor_tensor(
                        out=T_e[:, 0:w, :], in0=T_e[:, 0:w, :],
                        in1=T_e[:, w : 2 * w, :], op=OP.add)

                # ---- mu_x = alpha*mu - beta*acc_e (acc_e already includes b2)
                mx = php.tile([128, HF], F32)
                nc.vector.tensor_scalar_mul(out=mx, in0=muh, scalar1=alpha)
                nc.vector.scalar_tensor_tensor(
                    out=mx, in0=T_e[:, 0, :], scalar=negbeta, in1=mx,
                    op0=OP.mult, op1=OP.add)

                # ---- args a_i = (e_i - mu_x)*inv via 3 anchors + stepping
                st = php.tile([128, HF], F16)
                nc.vector.tensor_scalar_mul(out=st, in0=inv, scalar1=0.125)
                ats = {}
                for i in (3, 8, 13):
                    a = fpool.tile([128, HF], F16)
                    nc.vector.tensor_scalar(
                        out=a, in0=mx, scalar1=-1.0,
                        scalar2=float(i / 8.0 - 1.0), op0=OP.mult, op1=OP.add)
                    nc.vector.tensor_mul(out=a, in0=a, in1=inv)
                    ats[i] = a
                for src_i, dst_i in ((3, 2), (2, 1), (3, 4), (4, 5),
                                     (8, 7), (7, 6), (8, 9), (9, 10),
                                     (13, 12), (12, 11), (13, 14), (14, 15)):
                    a = fpool.tile([128, HF], F16)
                    nc.vector.tensor_tensor(
                        out=a, in0=ats[src_i], in1=st,
                        op=OP.add if dst_i > src_i else OP.subtract)
                    ats[dst_i] = a
                # f_i = 0.5*erf(a_i), in place
                fts = []
                for i in range(1, 16):
                    fi = ats[i]
                    nc.scalar.activation(out=fi, in_=fi, func=AF.Erf)
                    nc.vector.tensor_scalar_mul(out=fi, in0=fi, scalar1=0.5)
                    fts.append(fi)

                # ---- diffs: contiguous per-k tiles, cast-DMA each k-plane
                for k2 in range(K):
                    o = opool.tile([128, HF], F16)
                    if k2 == 0:
                        nc.vector.tensor_scalar_add(
                            out=o, in0=fts[0], scalar1=0.5)
                    elif k2 == 15:
                        nc.vector.tensor_scalar(
                            out=o, in0=fts[14], scalar1=-1.0, scalar2=0.5,
                            op0=OP.mult, op1=OP.add)
                    else:
                        nc.vector.tensor_tensor(
                            out=o, in0=fts[k2], in1=fts[k2 - 1],
                            op=OP.subtract)
                    nc.gpsimd.dma_start(out=out_p[:, k2, sl], in_=o[:, :])

    return nc


def _host_consts(t, W1, b1, W2, b2):
    t = np.asarray(t, np.float64).reshape(B)
    cond = t < T_MIN
    gamma = 1.0 - SIGMA_ONE ** (2.0 * t)
    alpha = np.where(cond, 0.0, 1.0 / gamma)
    beta = np.sqrt(np.maximum(1.0 - gamma, 0.0) / gamma)
    negbeta = np.where(cond, 0.0, -beta)
    lnA = np.where(cond, -1e4, np.log(np.maximum(beta, 1e-300)))
    lnm = np.where(cond, 0.0, np.log(SIGMA_ONE))
    nb20 = np.where(cond, 0.0, -beta * float(b2[0]))

    pb = np.zeros((128, 8), np.float32)
    for b in range(B):
        for q in range(Q):
            p = b * Q + q
            pb[p, 0] = alpha[b]
            pb[p, 1] = negbeta[b]
            pb[p, 2] = lnA[b]
            pb[p, 3] = lnm[b]
            pb[p, 4] = nb20[b]
            pb[p, 5] = -LN_SQRT2

    cb = np.zeros((128, H), np.float32)
    cvals = t[:, None] * np.asarray(W1, np.float64)[1, :][None, :] + np.asarray(
        b1, np.float64)[None, :]                        # [B, H]
    for b in range(B):
        cb[b * Q : (b + 1) * Q, :] = cvals[b]
    return cb, pb


def _run(inputs, trace=False):
    mu = np.ascontiguousarray(np.asarray(inputs["mu"], np.float32))
    t = np.asarray(inputs["t"], np.float32)
    W1 = np.asarray(inputs["W1"], np.float32)
    b1 = np.asarray(inputs["b1"], np.float32)
    W2 = np.asarray(inputs["W2"], np.float32)
    b2 = np.asarray(inputs["b2"], np.float32)

    nc = _build(W1, b1, W2, b2)
    nc.finalize()
    cb, pb = _host_consts(t, W1, b1, W2, b2)

    cn = np.ascontiguousarray(np.concatenate([cb, pb], axis=1))
    in_maps = []
    for c in range(NCORES):
        shard = np.ascontiguousarray(mu[:, c * DS : (c + 1) * DS])
        in_maps.append({"mu": shard, "cn": cn})

    res = run_bass_kernel_spmd(nc, in_maps, list(range(NCORES)), trace=trace)
    shards = []
    for c in range(NCORES):
        s = np.asarray(res.results[c]["out"])          # [128, K, F]
        shards.append(s.reshape(B, Q, K, F).transpose(0, 1, 3, 2).reshape(B, DS, K))
    out = np.ascontiguousarray(np.concatenate(shards, axis=1))
    return out, res


def kernel(**inputs) -> np.ndarray:
    out, _ = _run(inputs, trace=False)
    return out


if __name__ == "__main__":
    rng = np.random.default_rng(0)
    demo = {
        "mu": rng.standard_normal((B, D), dtype=np.float32),
        "t": rng.random((B, 1), dtype=np.float32),
        "W1": rng.standard_normal((2, H), dtype=np.float32) * 0.5,
        "b1": rng.standard_normal((H,), dtype=np.float32) * 0.1,
        "W2": rng.standard_normal((H, 2), dtype=np.float32) * 0.1,
        "b2": rng.standard_normal((2,), dtype=np.float32) * 0.1,
    }
    out = kernel(**demo)
    print("kernel output", out.shape, out.dtype, out[0, 0])
